# revision 5
# baseline (speedup 1.0000x reference)
"""CrossMerge kernel for trn2 — v4 (DMA-roofline oriented).

Math (per batch element):
    means_i = mean over C of g_i              (4, H, W)
    logits  = w_proj @ means + b_proj         (4, H, W)
    w       = softmax(logits, axis=1)         (4, H, W)
    out     = sum_i g_i * w_i                 (C, H, W)

Sharding: data-parallel over batch B=8 across 8 cores; weights replicated;
no cross-device communication.

Structure: 9 blocks of 1024 spatial columns. Each block is loaded with two
2.1MB SWDGE DMAs that cast fp32->bf16 in flight (HBM traffic unchanged --
the binding resource: 47.2MB @ ~370GB/s ~= 130us). Per 512-col slice the
softmax chain runs on PE (8 logits MMs, 1 denom) / ACT (exp) / DVE
(reciprocal, W=E*R) / PE (4 broadcast MMs) / ACT (PSUM->SBUF bf16 copies
into a 1024-wide weight tile). Products + accumulation then run once per
block as contiguous 2-dim [128,1024] bf16 tensor_tensor ops (DVE 2x mode;
3-dim or 0-stride APs drop to 1x -- measured), with the grid-3 product and
the q2+q3 add offloaded to GpSimd. Final adds write fp32 into the store
tile; stores go on the sync/HWDGE queue so they never head-block the load
queue. Loads are emitted two blocks ahead so the Pool-queue FIFO keeps the
SDMA engines fed while GpSimd chews on product work.

Tolerance is rel_err < 2e-2; bf16 internals land ~4e-3.
"""

import os
import sys
from contextlib import ExitStack

import numpy as np

try:
    import concourse.bass as bass
except ImportError:  # fresh grading dir: concourse lives in the container repo
    sys.path.insert(0, "/opt/trn_rl_repo")
    import concourse.bass as bass

import concourse.tile as tile
from concourse import bacc, mybir
from concourse.bass_utils import run_bass_kernel_spmd

B, C, H, W = 8, 256, 96, 96
HW = H * W  # 9216
NCORES = 8
CPB = C // 128  # 2 partition chunks per core
JCOLS = 512  # softmax slice width (= fp32 PSUM bank)
BCOLS = 1024  # product/store block width
NBLK = HW // BCOLS  # 9

F32 = mybir.dt.float32
BF16 = mybir.dt.bfloat16
U16 = mybir.dt.uint16
AF = mybir.ActivationFunctionType

_CACHE = {}


def build_program():
    nc = bacc.Bacc("TRN2", debug=False, num_devices=NCORES)

    gall_d = nc.dram_tensor("gall", [4, C, HW], F32, kind="ExternalInput").ap()
    # bf16 constants, one blob: 0-15 ws | 16-19 ones4x4 | 20-531 selmat
    cbu_d = nc.dram_tensor("cbu", [128, 532], U16, kind="ExternalInput").ap()
    # fp32 constants: col 0 = exp bias (rows 0-3)
    cf_d = nc.dram_tensor("cf", [128, 1], F32, kind="ExternalInput").ap()
    out = nc.dram_tensor("out", [C, HW], F32, kind="ExternalOutput").ap()

    with tile.TileContext(nc) as tc, ExitStack() as ctx:
        const = ctx.enter_context(tc.tile_pool(name="const", bufs=1))
        gin = ctx.enter_context(tc.tile_pool(name="gin", bufs=4))
        outp = ctx.enter_context(tc.tile_pool(name="outp", bufs=3))
        narrow = ctx.enter_context(tc.tile_pool(name="narrow", bufs=3))
        wbsb = ctx.enter_context(tc.tile_pool(name="wbsb", bufs=2))
        prod = ctx.enter_context(tc.tile_pool(name="prod", bufs=2))
        ps_L = ctx.enter_context(tc.tile_pool(name="psL", bufs=2, space="PSUM"))
        ps_S4 = ctx.enter_context(tc.tile_pool(name="psS4", bufs=2, space="PSUM"))
        ps_Wb = ctx.enter_context(tc.tile_pool(name="psWb", bufs=1, space="PSUM"))

        cbu = const.tile([128, 532], U16)
        nc.sync.dma_start(out=cbu[:], in_=cbu_d)
        cb = cbu.bitcast(BF16)
        ws = cb[:, 0:16]
        ones4x4 = cb[0:4, 16:20]
        selmat = cb[0:4, 20:532]
        cf = const.tile([128, 1], F32)
        nc.sync.dma_start(out=cf[:], in_=cf_d)
        bv = cf[0:4, 0:1]

        gats = {}

        def emit_load(b):
            if b >= NBLK:
                return
            gat = gin.tile([128, 4, CPB, BCOLS], BF16, tag="gall")
            for j in range(2):
                n0 = b * BCOLS + j * JCOLS
                nc.gpsimd.dma_start(
                    out=gat[:, :, :, j * JCOLS : (j + 1) * JCOLS],
                    in_=gall_d[:, :, n0 : n0 + JCOLS].rearrange(
                        "i (c p) n -> p i c n", c=CPB
                    ),
                )
            gats[b] = gat

        def softmax_slice(gat, j, wbs):
            """Chain for 512-col slice j; writes bf16 weights into wbs[i]."""
            x = slice(j * JCOLS, (j + 1) * JCOLS)
            L = ps_L.tile([4, JCOLS], F32, tag="L")
            k = 0
            for i in range(4):
                for c in range(CPB):
                    nc.tensor.matmul(
                        L,
                        lhsT=ws[:, 4 * i : 4 * i + 4],
                        rhs=gat[:, i, c, x],
                        start=(k == 0),
                        stop=(k == 7),
                    )
                    k += 1
            E = narrow.tile([4, JCOLS], BF16, tag="E")
            nc.scalar.activation(E[:], L, AF.Exp, bias=bv, scale=1.0)
            S4 = ps_S4.tile([4, JCOLS], F32, tag="S4")
            nc.tensor.matmul(S4[:], lhsT=ones4x4, rhs=E[:], start=True, stop=True)
            # reciprocal DVE op requires base partition 0 (HW-verified in v1)
            R4 = narrow.tile([4, JCOLS], F32, tag="R4")
            nc.vector.reciprocal_approx_fast(R4[:], S4[:])
            W4 = narrow.tile([4, JCOLS], BF16, tag="W4")
            nc.vector.tensor_mul(W4[:], E[:], R4[:])
            for i in range(4):
                Wbp = ps_Wb.tile([128, JCOLS], F32, tag=f"wb{i}")
                nc.tensor.matmul(
                    Wbp[:],
                    lhsT=selmat[:, 128 * i : 128 * (i + 1)],
                    rhs=W4[:],
                    start=True,
                    stop=True,
                )
                nc.scalar.copy(wbs[i][:, x], Wbp[:])

        emit_load(0)
        emit_load(1)
        for b in range(NBLK):
            emit_load(b + 2)
            gat = gats.pop(b)
            wbs = [
                wbsb.tile([128, BCOLS], BF16, tag=f"ws{i}", name=f"wbs{i}")
                for i in range(4)
            ]
            for j in range(2):
                softmax_slice(gat, j, wbs)
            ot = outp.tile([128, CPB, BCOLS], F32, tag="ot")
            # products + accumulation: contiguous [128,1024] bf16 ops
            for c in range(CPB):
                q0 = prod.tile([128, BCOLS], BF16, tag="q0")
                nc.vector.tensor_mul(q0[:], gat[:, 0, c, :], wbs[0][:])
                q1 = prod.tile([128, BCOLS], BF16, tag="q1")
                nc.vector.tensor_mul(q1[:], gat[:, 1, c, :], wbs[1][:])
                s01 = prod.tile([128, BCOLS], BF16, tag="s01")
                nc.vector.tensor_add(s01[:], q0[:], q1[:])
                q2 = prod.tile([128, BCOLS], BF16, tag="q2")
                nc.vector.tensor_mul(q2[:], gat[:, 2, c, :], wbs[2][:])
                q3 = prod.tile([128, BCOLS], BF16, tag="q3")
                nc.gpsimd.tensor_mul(q3[:], gat[:, 3, c, :], wbs[3][:])
                s23 = prod.tile([128, BCOLS], BF16, tag="s23")
                nc.gpsimd.tensor_add(s23[:], q2[:], q3[:])
                nc.vector.tensor_add(ot[:, c, :], s01[:], s23[:])
            N0 = b * BCOLS
            nc.sync.dma_start(
                out=out[:, N0 : N0 + BCOLS].rearrange("(c p) n -> p c n", c=CPB),
                in_=ot[:],
            )

    nc.compile()
    return nc


def _get_program():
    if "nc" not in _CACHE:
        _CACHE["nc"] = build_program()
    return _CACHE["nc"]


def _to_bf16_bits(x):
    """Round-to-nearest-even fp32 -> bf16 bit pattern (uint16)."""
    u = np.asarray(x, dtype=np.float32).view(np.uint32)
    rounded = u + 0x7FFF + ((u >> 16) & 1)
    return (rounded >> 16).astype(np.uint16)


def make_consts(w_proj, b_proj):
    w = np.asarray(w_proj, dtype=np.float32)
    b = np.asarray(b_proj, dtype=np.float32)
    ws = np.empty((128, 16), dtype=np.float32)
    for i in range(4):
        for o in range(4):
            ws[:, 4 * i + o] = w[o, i] / C
    cbu = np.zeros((128, 532), dtype=np.float32)
    cbu[:, 0:16] = ws
    cbu[0:4, 16:20] = 1.0
    cbu[0:4, 20:532] = np.repeat(np.eye(4, dtype=np.float32), 128, axis=1)
    cf = np.zeros((128, 1), dtype=np.float32)
    cf[0:4, 0] = b
    return _to_bf16_bits(cbu), cf


LAST_RESULT = None


def kernel(g0, g1, g2, g3, w_proj, b_proj):
    global LAST_RESULT
    nc = _get_program()

    cbu, cf = make_consts(w_proj, b_proj)

    gall = np.stack(
        [np.asarray(x, dtype=np.float32).reshape(B, C, HW) for x in (g0, g1, g2, g3)],
        axis=1,
    )  # (B, 4, C, HW)
    in_maps = []
    for bi in range(NCORES):
        m = {"gall": np.ascontiguousarray(gall[bi]), "cbu": cbu, "cf": cf}
        in_maps.append(m)

    res = run_bass_kernel_spmd(
        nc,
        in_maps,
        list(range(NCORES)),
        trace=bool(int(os.environ.get("CM_TRACE", "0"))),
        tmpdir=os.environ.get("CM_TRACE_DIR") or None,
    )
    LAST_RESULT = res
    out_full = np.stack(
        [res.results[bi]["out"].reshape(C, H, W) for bi in range(NCORES)], axis=0
    )
    return out_full


# revision 6
# speedup vs baseline: 1.2438x; 1.2438x over previous
"""CrossMerge kernel for trn2 — v4 (DMA-roofline oriented).

Math (per batch element):
    means_i = mean over C of g_i              (4, H, W)
    logits  = w_proj @ means + b_proj         (4, H, W)
    w       = softmax(logits, axis=1)         (4, H, W)
    out     = sum_i g_i * w_i                 (C, H, W)

Sharding: data-parallel over batch B=8 across 8 cores; weights replicated;
no cross-device communication.

Structure: 9 blocks of 1024 spatial columns. Each block is loaded with two
2.1MB SWDGE DMAs that cast fp32->bf16 in flight (HBM traffic unchanged --
the binding resource: 47.2MB @ ~370GB/s ~= 130us). Per 512-col slice the
softmax chain runs on PE (8 logits MMs, 1 denom) / ACT (exp) / DVE
(reciprocal, W=E*R) / PE (4 broadcast MMs) / ACT (PSUM->SBUF bf16 copies
into a 1024-wide weight tile). Products + accumulation then run once per
block as contiguous 2-dim [128,1024] bf16 tensor_tensor ops (DVE 2x mode;
3-dim or 0-stride APs drop to 1x -- measured), with the grid-3 product and
the q2+q3 add offloaded to GpSimd. Final adds write fp32 into the store
tile; stores go on the sync/HWDGE queue so they never head-block the load
queue. Loads are emitted two blocks ahead so the Pool-queue FIFO keeps the
SDMA engines fed while GpSimd chews on product work.

Tolerance is rel_err < 2e-2; bf16 internals land ~4e-3.
"""

import os
import sys
from contextlib import ExitStack

import numpy as np

try:
    import concourse.bass as bass
except ImportError:  # fresh grading dir: concourse lives in the container repo
    sys.path.insert(0, "/opt/trn_rl_repo")
    import concourse.bass as bass

import concourse.tile as tile
from concourse import bacc, mybir
from concourse.bass_utils import run_bass_kernel_spmd

B, C, H, W = 8, 256, 96, 96
HW = H * W  # 9216
NCORES = 8
CPB = C // 128  # 2 partition chunks per core
JCOLS = 512  # softmax slice width (= fp32 PSUM bank)
BCOLS = 1024  # product/store block width
NBLK = HW // BCOLS  # 9

F32 = mybir.dt.float32
BF16 = mybir.dt.bfloat16
U16 = mybir.dt.uint16
AF = mybir.ActivationFunctionType

_CACHE = {}


def build_program():
    nc = bacc.Bacc("TRN2", debug=False, num_devices=NCORES)

    gall_d = nc.dram_tensor("gall", [4, C, HW], F32, kind="ExternalInput").ap()
    # bf16 constants, one blob: 0-15 ws | 16-19 ones4x4 | 20-531 selmat
    cbu_d = nc.dram_tensor("cbu", [128, 532], U16, kind="ExternalInput").ap()
    # fp32 constants: col 0 = exp bias (rows 0-3)
    cf_d = nc.dram_tensor("cf", [128, 1], F32, kind="ExternalInput").ap()
    out = nc.dram_tensor("out", [C, HW], F32, kind="ExternalOutput").ap()

    with tile.TileContext(nc) as tc, ExitStack() as ctx:
        const = ctx.enter_context(tc.tile_pool(name="const", bufs=1))
        gin = ctx.enter_context(tc.tile_pool(name="gin", bufs=4))
        outp = ctx.enter_context(tc.tile_pool(name="outp", bufs=3))
        narrow = ctx.enter_context(tc.tile_pool(name="narrow", bufs=3))
        wbsb = ctx.enter_context(tc.tile_pool(name="wbsb", bufs=2))
        prod = ctx.enter_context(tc.tile_pool(name="prod", bufs=2))
        ps_L = ctx.enter_context(tc.tile_pool(name="psL", bufs=2, space="PSUM"))
        ps_S4 = ctx.enter_context(tc.tile_pool(name="psS4", bufs=2, space="PSUM"))
        ps_Wb = ctx.enter_context(tc.tile_pool(name="psWb", bufs=1, space="PSUM"))

        cbu = const.tile([128, 532], U16)
        nc.sync.dma_start(out=cbu[:], in_=cbu_d)
        cb = cbu.bitcast(BF16)
        ws = cb[:, 0:16]
        ones4x4 = cb[0:4, 16:20]
        selmat = cb[0:4, 20:532]
        cf = const.tile([128, 1], F32)
        nc.sync.dma_start(out=cf[:], in_=cf_d)
        bv = cf[0:4, 0:1]

        gats = {}

        def emit_load(b):
            if b >= NBLK:
                return
            gat = gin.tile([128, 4, CPB, BCOLS], BF16, tag="gall")
            for j in range(2):
                n0 = b * BCOLS + j * JCOLS
                nc.gpsimd.dma_start(
                    out=gat[:, :, :, j * JCOLS : (j + 1) * JCOLS],
                    in_=gall_d[:, :, n0 : n0 + JCOLS].rearrange(
                        "i (c p) n -> p i c n", c=CPB
                    ),
                )
            gats[b] = gat

        def softmax_slice(gat, j, wbs):
            """Chain for 512-col slice j; writes bf16 weights into wbs[i]."""
            x = slice(j * JCOLS, (j + 1) * JCOLS)
            L = ps_L.tile([4, JCOLS], F32, tag="L")
            k = 0
            for i in range(4):
                for c in range(CPB):
                    nc.tensor.matmul(
                        L,
                        lhsT=ws[:, 4 * i : 4 * i + 4],
                        rhs=gat[:, i, c, x],
                        start=(k == 0),
                        stop=(k == 7),
                    )
                    k += 1
            E = narrow.tile([4, JCOLS], BF16, tag="E")
            nc.scalar.activation(E[:], L, AF.Exp, bias=bv, scale=1.0)
            S4 = ps_S4.tile([4, JCOLS], F32, tag="S4")
            nc.tensor.matmul(S4[:], lhsT=ones4x4, rhs=E[:], start=True, stop=True)
            # reciprocal DVE op requires base partition 0 (HW-verified in v1)
            R4 = narrow.tile([4, JCOLS], F32, tag="R4")
            nc.vector.reciprocal_approx_fast(R4[:], S4[:])
            W4 = narrow.tile([4, JCOLS], BF16, tag="W4")
            nc.vector.tensor_mul(W4[:], E[:], R4[:])
            for i in range(4):
                Wbp = ps_Wb.tile([128, JCOLS], F32, tag=f"wb{i}")
                nc.tensor.matmul(
                    Wbp[:],
                    lhsT=selmat[:, 128 * i : 128 * (i + 1)],
                    rhs=W4[:],
                    start=True,
                    stop=True,
                )
                nc.scalar.copy(wbs[i][:, x], Wbp[:])

        emit_load(0)
        emit_load(1)
        for b in range(NBLK):
            emit_load(b + 2)
            gat = gats.pop(b)
            wbs = [
                wbsb.tile([128, BCOLS], BF16, tag=f"ws{i}", name=f"wbs{i}")
                for i in range(4)
            ]
            for j in range(2):
                softmax_slice(gat, j, wbs)
            ot = outp.tile([128, CPB, BCOLS], F32, tag="ot")
            # products + accumulation: contiguous [128,1024] bf16 ops
            for c in range(CPB):
                q0 = prod.tile([128, BCOLS], BF16, tag="q0")
                nc.vector.tensor_mul(q0[:], gat[:, 0, c, :], wbs[0][:])
                q1 = prod.tile([128, BCOLS], BF16, tag="q1")
                nc.vector.tensor_mul(q1[:], gat[:, 1, c, :], wbs[1][:])
                s01 = prod.tile([128, BCOLS], BF16, tag="s01")
                nc.vector.tensor_add(s01[:], q0[:], q1[:])
                q2 = prod.tile([128, BCOLS], BF16, tag="q2")
                nc.vector.tensor_mul(q2[:], gat[:, 2, c, :], wbs[2][:])
                q3 = prod.tile([128, BCOLS], BF16, tag="q3")
                nc.gpsimd.tensor_mul(q3[:], gat[:, 3, c, :], wbs[3][:])
                s23 = prod.tile([128, BCOLS], BF16, tag="s23")
                nc.vector.tensor_add(s23[:], q2[:], q3[:])
                nc.vector.tensor_add(ot[:, c, :], s01[:], s23[:])
            N0 = b * BCOLS
            nc.sync.dma_start(
                out=out[:, N0 : N0 + BCOLS].rearrange("(c p) n -> p c n", c=CPB),
                in_=ot[:],
            )

    nc.compile()
    return nc


def _get_program():
    if "nc" not in _CACHE:
        _CACHE["nc"] = build_program()
    return _CACHE["nc"]


def _to_bf16_bits(x):
    """Round-to-nearest-even fp32 -> bf16 bit pattern (uint16)."""
    u = np.asarray(x, dtype=np.float32).view(np.uint32)
    rounded = u + 0x7FFF + ((u >> 16) & 1)
    return (rounded >> 16).astype(np.uint16)


def make_consts(w_proj, b_proj):
    w = np.asarray(w_proj, dtype=np.float32)
    b = np.asarray(b_proj, dtype=np.float32)
    ws = np.empty((128, 16), dtype=np.float32)
    for i in range(4):
        for o in range(4):
            ws[:, 4 * i + o] = w[o, i] / C
    cbu = np.zeros((128, 532), dtype=np.float32)
    cbu[:, 0:16] = ws
    cbu[0:4, 16:20] = 1.0
    cbu[0:4, 20:532] = np.repeat(np.eye(4, dtype=np.float32), 128, axis=1)
    cf = np.zeros((128, 1), dtype=np.float32)
    cf[0:4, 0] = b
    return _to_bf16_bits(cbu), cf


LAST_RESULT = None


def kernel(g0, g1, g2, g3, w_proj, b_proj):
    global LAST_RESULT
    nc = _get_program()

    cbu, cf = make_consts(w_proj, b_proj)

    gall = np.stack(
        [np.asarray(x, dtype=np.float32).reshape(B, C, HW) for x in (g0, g1, g2, g3)],
        axis=1,
    )  # (B, 4, C, HW)
    in_maps = []
    for bi in range(NCORES):
        m = {"gall": np.ascontiguousarray(gall[bi]), "cbu": cbu, "cf": cf}
        in_maps.append(m)

    res = run_bass_kernel_spmd(
        nc,
        in_maps,
        list(range(NCORES)),
        trace=bool(int(os.environ.get("CM_TRACE", "0"))),
        tmpdir=os.environ.get("CM_TRACE_DIR") or None,
    )
    LAST_RESULT = res
    out_full = np.stack(
        [res.results[bi]["out"].reshape(C, H, W) for bi in range(NCORES)], axis=0
    )
    return out_full


# revision 7
# speedup vs baseline: 1.3442x; 1.0807x over previous
"""CrossMerge kernel for trn2 — v5 (software-pipelined emission).

Math (per batch element):
    means_i = mean over C of g_i              (4, H, W)
    logits  = w_proj @ means + b_proj         (4, H, W)
    w       = softmax(logits, axis=1)         (4, H, W)
    out     = sum_i g_i * w_i                 (C, H, W)

Sharding: data-parallel over batch B=8 across 8 cores; weights replicated.

HBM traffic is the binding resource: 47.2MB @ ~370GB/s ~= 130us. All
engine queues are in-order, so the per-slice softmax chain (PE logits ->
ACT exp -> PE denom -> DVE recip/W4 -> PE bcast -> ACT copy -> DVE/Pool
products) head-blocks every engine if emitted naively (v4 measured the PE
at 1.2GHz cold-clock with 17-52us HAM-cold stretches caused exactly by
those stalls). v5 staggers the emission so every engine's stream has its
inputs ready by the time the instruction reaches the queue head:

    round s:  load(block s/2+2)      [Pool/SWDGE queue, fp32->bf16 cast]
              P1: logits(s)          [PE, needs only gat]
              A1: exp(s-1)           [ACT]
              P2: denom(s-1)         [PE]
              V:  recip+W4(s-2)      [DVE]
              P3: bcast x4 (s-2)     [PE]
              A2: copies x4 (s-2)    [ACT]
              products(block) on odd rounds, 2 rounds behind  [DVE + Pool]
              store(block)           [Pool/SWDGE queue, bf16->fp32 cast]

Products are contiguous 2-dim [128,1024] bf16 tensor_tensor ops (DVE 2x
mode; 3-dim or 0-stride APs drop to 1x -- measured). Grid-3 products run
on GpSimd. The store tile is bf16 (final adds keep 2x) and SWDGE upcasts
on the way out; loads are emitted two blocks ahead so the Pool-queue FIFO
keeps the SDMA engines fed.

Tolerance is rel_err < 2e-2; bf16 internals land ~4e-3.
"""

import os
import sys
from contextlib import ExitStack

import numpy as np

try:
    import concourse.bass as bass
except ImportError:  # fresh grading dir: concourse lives in the container repo
    sys.path.insert(0, "/opt/trn_rl_repo")
    import concourse.bass as bass

import concourse.tile as tile
from concourse import bacc, mybir
from concourse.bass_utils import run_bass_kernel_spmd

B, C, H, W = 8, 256, 96, 96
HW = H * W  # 9216
NCORES = 8
CPB = C // 128  # 2 partition chunks per core
JCOLS = 512  # softmax slice width (= fp32 PSUM bank)
NSL = HW // JCOLS  # 18 slices
BCOLS = 1024  # product/store block width
NBLK = HW // BCOLS  # 9

F32 = mybir.dt.float32
BF16 = mybir.dt.bfloat16
U16 = mybir.dt.uint16
AF = mybir.ActivationFunctionType

_CACHE = {}


def build_program():
    nc = bacc.Bacc("TRN2", debug=False, num_devices=NCORES)

    gall_d = nc.dram_tensor("gall", [4, C, HW], F32, kind="ExternalInput").ap()
    # bf16 constants, one blob: 0-15 ws | 16-19 ones4x4 | 20-531 selmat
    cbu_d = nc.dram_tensor("cbu", [128, 532], U16, kind="ExternalInput").ap()
    # fp32 constants: col 0 = exp bias (rows 0-3)
    cf_d = nc.dram_tensor("cf", [128, 1], F32, kind="ExternalInput").ap()
    out = nc.dram_tensor("out", [C, HW], F32, kind="ExternalOutput").ap()

    with tile.TileContext(nc) as tc, ExitStack() as ctx:
        const = ctx.enter_context(tc.tile_pool(name="const", bufs=1))
        gin = ctx.enter_context(tc.tile_pool(name="gin", bufs=5))
        outp = ctx.enter_context(tc.tile_pool(name="outp", bufs=3))
        narrow = ctx.enter_context(tc.tile_pool(name="narrow", bufs=4))
        wbsb = ctx.enter_context(tc.tile_pool(name="wbsb", bufs=3))
        prod = ctx.enter_context(tc.tile_pool(name="prod", bufs=2))
        ps_L = ctx.enter_context(tc.tile_pool(name="psL", bufs=2, space="PSUM"))
        ps_S4 = ctx.enter_context(tc.tile_pool(name="psS4", bufs=2, space="PSUM"))
        ps_Wb = ctx.enter_context(tc.tile_pool(name="psWb", bufs=1, space="PSUM"))

        cbu = const.tile([128, 532], U16)
        nc.sync.dma_start(out=cbu[:], in_=cbu_d)
        cb = cbu.bitcast(BF16)
        ws = cb[:, 0:16]
        ones4x4 = cb[0:4, 16:20]
        selmat = cb[0:4, 20:532]
        cf = const.tile([128, 1], F32)
        nc.sync.dma_start(out=cf[:], in_=cf_d)
        bv = cf[0:4, 0:1]

        gats = {}  # block -> gat tile
        st = {}  # slice -> dict of live tiles

        def emit_load(b):
            if not (0 <= b < NBLK):
                return
            gat = gin.tile([128, 4, CPB, BCOLS], BF16, tag="gall")
            for j in range(2):
                n0 = b * BCOLS + j * JCOLS
                nc.gpsimd.dma_start(
                    out=gat[:, :, :, j * JCOLS : (j + 1) * JCOLS],
                    in_=gall_d[:, :, n0 : n0 + JCOLS].rearrange(
                        "i (c p) n -> p i c n", c=CPB
                    ),
                )
            gats[b] = gat

        def xsl(s):
            return slice((s % 2) * JCOLS, (s % 2 + 1) * JCOLS)

        def p1_logits(s):
            if not (0 <= s < NSL):
                return
            gat = gats[s // 2]
            L = ps_L.tile([4, JCOLS], F32, tag="L")
            k = 0
            for i in range(4):
                for c in range(CPB):
                    nc.tensor.matmul(
                        L,
                        lhsT=ws[:, 4 * i : 4 * i + 4],
                        rhs=gat[:, i, c, xsl(s)],
                        start=(k == 0),
                        stop=(k == 7),
                    )
                    k += 1
            st[s] = {"L": L}

        def a1_exp(s):
            if not (0 <= s < NSL):
                return
            E = narrow.tile([4, JCOLS], BF16, tag="E")
            nc.scalar.activation(E[:], st[s]["L"], AF.Exp, bias=bv, scale=1.0)
            st[s]["E"] = E

        def p2_denom(s):
            if not (0 <= s < NSL):
                return
            S4 = ps_S4.tile([4, JCOLS], F32, tag="S4")
            nc.tensor.matmul(
                S4[:], lhsT=ones4x4, rhs=st[s]["E"][:], start=True, stop=True
            )
            st[s]["S4"] = S4

        def v_weights(s):
            if not (0 <= s < NSL):
                return
            # reciprocal DVE op requires base partition 0 (HW-verified in v1)
            R4 = narrow.tile([4, JCOLS], F32, tag="R4")
            nc.vector.reciprocal_approx_fast(R4[:], st[s]["S4"][:])
            W4 = narrow.tile([4, JCOLS], BF16, tag="W4")
            nc.vector.tensor_mul(W4[:], st[s]["E"][:], R4[:])
            st[s]["W4"] = W4

        def p3_bcast(s):
            if not (0 <= s < NSL):
                return
            wbps = []
            for i in range(4):
                Wbp = ps_Wb.tile([128, JCOLS], F32, tag=f"wb{i}")
                nc.tensor.matmul(
                    Wbp[:],
                    lhsT=selmat[:, 128 * i : 128 * (i + 1)],
                    rhs=st[s]["W4"][:],
                    start=True,
                    stop=True,
                )
                wbps.append(Wbp)
            st[s]["wbp"] = wbps

        def a2_copies(s, wbs_by_block):
            if not (0 <= s < NSL):
                return
            b = s // 2
            if b not in wbs_by_block:
                wbs_by_block[b] = [
                    wbsb.tile([128, BCOLS], BF16, tag=f"ws{i}", name=f"wbs{i}")
                    for i in range(4)
                ]
            for i in range(4):
                nc.scalar.copy(wbs_by_block[b][i][:, xsl(s)], st[s]["wbp"][i][:])

        wbs_by_block = {}

        def products_store(b):
            if not (0 <= b < NBLK):
                return
            gat = gats.pop(b)
            wbs = wbs_by_block.pop(b)
            ot = outp.tile([128, CPB, BCOLS], BF16, tag="ot")
            for c in range(CPB):
                q0 = prod.tile([128, BCOLS], BF16, tag="q0")
                nc.vector.tensor_mul(q0[:], gat[:, 0, c, :], wbs[0][:])
                q1 = prod.tile([128, BCOLS], BF16, tag="q1")
                nc.vector.tensor_mul(q1[:], gat[:, 1, c, :], wbs[1][:])
                s01 = prod.tile([128, BCOLS], BF16, tag="s01")
                nc.vector.tensor_add(s01[:], q0[:], q1[:])
                q2 = prod.tile([128, BCOLS], BF16, tag="q2")
                nc.vector.tensor_mul(q2[:], gat[:, 2, c, :], wbs[2][:])
                q3 = prod.tile([128, BCOLS], BF16, tag="q3")
                nc.gpsimd.tensor_mul(q3[:], gat[:, 3, c, :], wbs[3][:])
                s23 = prod.tile([128, BCOLS], BF16, tag="s23")
                nc.vector.tensor_add(s23[:], q2[:], q3[:])
                nc.vector.tensor_add(ot[:, c, :], s01[:], s23[:])
            N0 = b * BCOLS
            # SWDGE cast-store: SBUF bf16 -> HBM fp32
            nc.gpsimd.dma_start(
                out=out[:, N0 : N0 + BCOLS].rearrange("(c p) n -> p c n", c=CPB),
                in_=ot[:],
            )

        emit_load(0)
        emit_load(1)
        for s in range(NSL + 4):
            if s % 2 == 0:
                emit_load(s // 2 + 2)
            p1_logits(s)
            a1_exp(s - 1)
            p2_denom(s - 1)
            v_weights(s - 2)
            p3_bcast(s - 2)
            a2_copies(s - 2, wbs_by_block)
            if s >= 3 and (s - 3) % 2 == 0:
                products_store((s - 3) // 2)
            # free stage state no longer needed
            st.pop(s - 3, None)

    nc.compile()
    return nc


def _get_program():
    if "nc" not in _CACHE:
        _CACHE["nc"] = build_program()
    return _CACHE["nc"]


def _to_bf16_bits(x):
    """Round-to-nearest-even fp32 -> bf16 bit pattern (uint16)."""
    u = np.asarray(x, dtype=np.float32).view(np.uint32)
    rounded = u + 0x7FFF + ((u >> 16) & 1)
    return (rounded >> 16).astype(np.uint16)


def make_consts(w_proj, b_proj):
    w = np.asarray(w_proj, dtype=np.float32)
    b = np.asarray(b_proj, dtype=np.float32)
    ws = np.empty((128, 16), dtype=np.float32)
    for i in range(4):
        for o in range(4):
            ws[:, 4 * i + o] = w[o, i] / C
    cbu = np.zeros((128, 532), dtype=np.float32)
    cbu[:, 0:16] = ws
    cbu[0:4, 16:20] = 1.0
    cbu[0:4, 20:532] = np.repeat(np.eye(4, dtype=np.float32), 128, axis=1)
    cf = np.zeros((128, 1), dtype=np.float32)
    cf[0:4, 0] = b
    return _to_bf16_bits(cbu), cf


LAST_RESULT = None


def kernel(g0, g1, g2, g3, w_proj, b_proj):
    global LAST_RESULT
    nc = _get_program()

    cbu, cf = make_consts(w_proj, b_proj)

    gall = np.stack(
        [np.asarray(x, dtype=np.float32).reshape(B, C, HW) for x in (g0, g1, g2, g3)],
        axis=1,
    )  # (B, 4, C, HW)
    in_maps = []
    for bi in range(NCORES):
        m = {"gall": np.ascontiguousarray(gall[bi]), "cbu": cbu, "cf": cf}
        in_maps.append(m)

    res = run_bass_kernel_spmd(
        nc,
        in_maps,
        list(range(NCORES)),
        trace=bool(int(os.environ.get("CM_TRACE", "0"))),
        tmpdir=os.environ.get("CM_TRACE_DIR") or None,
    )
    LAST_RESULT = res
    out_full = np.stack(
        [res.results[bi]["out"].reshape(C, H, W) for bi in range(NCORES)], axis=0
    )
    return out_full


# revision 10
# speedup vs baseline: 1.3587x; 1.0108x over previous
"""CrossMerge kernel for trn2 — v5 (software-pipelined emission).

Math (per batch element):
    means_i = mean over C of g_i              (4, H, W)
    logits  = w_proj @ means + b_proj         (4, H, W)
    w       = softmax(logits, axis=1)         (4, H, W)
    out     = sum_i g_i * w_i                 (C, H, W)

Sharding: data-parallel over batch B=8 across 8 cores; weights replicated.

HBM traffic is the binding resource: 47.2MB @ ~370GB/s ~= 130us. All
engine queues are in-order, so the per-slice softmax chain (PE logits ->
ACT exp -> PE denom -> DVE recip/W4 -> PE bcast -> ACT copy -> DVE/Pool
products) head-blocks every engine if emitted naively (v4 measured the PE
at 1.2GHz cold-clock with 17-52us HAM-cold stretches caused exactly by
those stalls). v5 staggers the emission so every engine's stream has its
inputs ready by the time the instruction reaches the queue head:

    round s:  load(block s/2+2)      [Pool/SWDGE queue, fp32->bf16 cast]
              P1: logits(s)          [PE, needs only gat]
              A1: exp(s-1)           [ACT]
              P2: denom(s-1)         [PE]
              V:  recip+W4(s-2)      [DVE]
              P3: bcast x4 (s-2)     [PE]
              A2: copies x4 (s-2)    [ACT]
              products(block) on odd rounds, 2 rounds behind  [DVE + Pool]
              store(block)           [Pool/SWDGE queue, bf16->fp32 cast]

Products are contiguous 2-dim [128,1024] bf16 tensor_tensor ops (DVE 2x
mode; 3-dim or 0-stride APs drop to 1x -- measured). Grid-3 products run
on GpSimd. The store tile is bf16 (final adds keep 2x) and SWDGE upcasts
on the way out; loads are emitted two blocks ahead so the Pool-queue FIFO
keeps the SDMA engines fed.

Tolerance is rel_err < 2e-2; bf16 internals land ~4e-3.
"""

import os
import sys
from contextlib import ExitStack

import numpy as np

try:
    import concourse.bass as bass
except ImportError:  # fresh grading dir: concourse lives in the container repo
    sys.path.insert(0, "/opt/trn_rl_repo")
    import concourse.bass as bass

import concourse.tile as tile
from concourse import bacc, mybir
from concourse.bass_utils import run_bass_kernel_spmd

B, C, H, W = 8, 256, 96, 96
HW = H * W  # 9216
NCORES = 8
CPB = C // 128  # 2 partition chunks per core
JCOLS = 512  # softmax slice width (= fp32 PSUM bank)
NSL = HW // JCOLS  # 18 slices
BCOLS = 1024  # product/store block width
NBLK = HW // BCOLS  # 9

F32 = mybir.dt.float32
BF16 = mybir.dt.bfloat16
U16 = mybir.dt.uint16
AF = mybir.ActivationFunctionType

_CACHE = {}


def build_program():
    nc = bacc.Bacc("TRN2", debug=False, num_devices=NCORES)

    gall_d = nc.dram_tensor("gall", [4, C, HW], F32, kind="ExternalInput").ap()
    # bf16 constants, one blob: 0-15 ws | 16-19 ones4x4 | 20-531 selmat
    cbu_d = nc.dram_tensor("cbu", [128, 532], U16, kind="ExternalInput").ap()
    # fp32 constants: col 0 = exp bias (rows 0-3)
    cf_d = nc.dram_tensor("cf", [128, 1], F32, kind="ExternalInput").ap()
    out = nc.dram_tensor("out", [C, HW], F32, kind="ExternalOutput").ap()

    with tile.TileContext(nc) as tc, ExitStack() as ctx:
        const = ctx.enter_context(tc.tile_pool(name="const", bufs=1))
        gin = ctx.enter_context(tc.tile_pool(name="gin", bufs=5))
        outp = ctx.enter_context(tc.tile_pool(name="outp", bufs=3))
        narrow = ctx.enter_context(tc.tile_pool(name="narrow", bufs=4))
        wbsb = ctx.enter_context(tc.tile_pool(name="wbsb", bufs=3))
        prod = ctx.enter_context(tc.tile_pool(name="prod", bufs=2))
        ps_L = ctx.enter_context(tc.tile_pool(name="psL", bufs=2, space="PSUM"))
        ps_S4 = ctx.enter_context(tc.tile_pool(name="psS4", bufs=2, space="PSUM"))
        ps_Wb = ctx.enter_context(tc.tile_pool(name="psWb", bufs=1, space="PSUM"))

        cbu = const.tile([128, 532], U16)
        nc.sync.dma_start(out=cbu[:], in_=cbu_d)
        cb = cbu.bitcast(BF16)
        ws = cb[:, 0:16]
        ones4x4 = cb[0:4, 16:20]
        selmat = cb[0:4, 20:532]
        cf = const.tile([128, 1], F32)
        nc.sync.dma_start(out=cf[:], in_=cf_d)
        bv = cf[0:4, 0:1]

        gats = {}  # block -> gat tile
        st = {}  # slice -> dict of live tiles

        def emit_load(b):
            if not (0 <= b < NBLK):
                return
            gat = gin.tile([128, 4, CPB, BCOLS], BF16, tag="gall")
            for j in range(2):
                n0 = b * BCOLS + j * JCOLS
                nc.gpsimd.dma_start(
                    out=gat[:, :, :, j * JCOLS : (j + 1) * JCOLS],
                    in_=gall_d[:, :, n0 : n0 + JCOLS].rearrange(
                        "i (c p) n -> p i c n", c=CPB
                    ),
                )
            gats[b] = gat

        def xsl(s):
            return slice((s % 2) * JCOLS, (s % 2 + 1) * JCOLS)

        def p1_logits(s):
            if not (0 <= s < NSL):
                return
            gat = gats[s // 2]
            L = ps_L.tile([4, JCOLS], F32, tag="L")
            k = 0
            for i in range(4):
                for c in range(CPB):
                    nc.tensor.matmul(
                        L,
                        lhsT=ws[:, 4 * i : 4 * i + 4],
                        rhs=gat[:, i, c, xsl(s)],
                        start=(k == 0),
                        stop=(k == 7),
                    )
                    k += 1
            st[s] = {"L": L}

        def a1_exp(s):
            if not (0 <= s < NSL):
                return
            E = narrow.tile([4, JCOLS], BF16, tag="E")
            nc.scalar.activation(E[:], st[s]["L"], AF.Exp, bias=bv, scale=1.0)
            st[s]["E"] = E

        def p2_denom(s):
            if not (0 <= s < NSL):
                return
            S4 = ps_S4.tile([4, JCOLS], F32, tag="S4")
            nc.tensor.matmul(
                S4[:], lhsT=ones4x4, rhs=st[s]["E"][:], start=True, stop=True
            )
            st[s]["S4"] = S4

        def v_weights(s):
            if not (0 <= s < NSL):
                return
            # reciprocal DVE op requires base partition 0 (HW-verified in v1)
            R4 = narrow.tile([4, JCOLS], F32, tag="R4")
            nc.vector.reciprocal_approx_fast(R4[:], st[s]["S4"][:])
            W4 = narrow.tile([4, JCOLS], BF16, tag="W4")
            # mixed bf16*f32 TT measured ~2.6us on DVE but ~1us on Pool
            nc.gpsimd.tensor_mul(W4[:], st[s]["E"][:], R4[:])
            st[s]["W4"] = W4

        def p3_bcast(s):
            if not (0 <= s < NSL):
                return
            wbps = []
            for i in range(4):
                Wbp = ps_Wb.tile([128, JCOLS], F32, tag=f"wb{i}")
                nc.tensor.matmul(
                    Wbp[:],
                    lhsT=selmat[:, 128 * i : 128 * (i + 1)],
                    rhs=st[s]["W4"][:],
                    start=True,
                    stop=True,
                )
                wbps.append(Wbp)
            st[s]["wbp"] = wbps

        def a2_copies(s, wbs_by_block):
            if not (0 <= s < NSL):
                return
            b = s // 2
            if b not in wbs_by_block:
                wbs_by_block[b] = [
                    wbsb.tile([128, BCOLS], BF16, tag=f"ws{i}", name=f"wbs{i}")
                    for i in range(4)
                ]
            for i in range(4):
                nc.scalar.copy(wbs_by_block[b][i][:, xsl(s)], st[s]["wbp"][i][:])

        wbs_by_block = {}
        ots = {}

        def products_chunk(b, c):
            if not (0 <= b < NBLK):
                return
            gat = gats[b]
            wbs = wbs_by_block[b]
            if c == 0:
                ots[b] = outp.tile([128, CPB, BCOLS], F32, tag="ot", name="ot")
            ot = ots[b]
            q0 = prod.tile([128, BCOLS], BF16, tag="q0")
            nc.vector.tensor_mul(q0[:], gat[:, 0, c, :], wbs[0][:])
            q1 = prod.tile([128, BCOLS], BF16, tag="q1")
            nc.vector.tensor_mul(q1[:], gat[:, 1, c, :], wbs[1][:])
            s01 = prod.tile([128, BCOLS], BF16, tag="s01")
            nc.vector.tensor_add(s01[:], q0[:], q1[:])
            q2 = prod.tile([128, BCOLS], BF16, tag="q2")
            nc.vector.tensor_mul(q2[:], gat[:, 2, c, :], wbs[2][:])
            q3 = prod.tile([128, BCOLS], BF16, tag="q3")
            nc.vector.tensor_mul(q3[:], gat[:, 3, c, :], wbs[3][:])
            s23 = prod.tile([128, BCOLS], BF16, tag="s23")
            nc.vector.tensor_add(s23[:], q2[:], q3[:])
            nc.vector.tensor_add(ot[:, c, :], s01[:], s23[:])
            if c == CPB - 1:
                gats.pop(b)
                wbs_by_block.pop(b)
                ot = ots.pop(b)
                N0 = b * BCOLS
                nc.sync.dma_start(
                    out=out[:, N0 : N0 + BCOLS].rearrange("(c p) n -> p c n", c=CPB),
                    in_=ot[:],
                )

        emit_load(0)
        emit_load(1)
        for s in range(NSL + 5):
            if s % 2 == 0:
                emit_load(s // 2 + 2)
            p1_logits(s)
            a1_exp(s - 1)
            p2_denom(s - 1)
            v_weights(s - 2)
            p3_bcast(s - 2)
            a2_copies(s - 2, wbs_by_block)
            if s >= 3:
                products_chunk((s - 3) // 2, (s - 3) % 2)
            # free stage state no longer needed
            st.pop(s - 3, None)

    nc.compile()
    return nc


def _get_program():
    if "nc" not in _CACHE:
        _CACHE["nc"] = build_program()
    return _CACHE["nc"]


def _to_bf16_bits(x):
    """Round-to-nearest-even fp32 -> bf16 bit pattern (uint16)."""
    u = np.asarray(x, dtype=np.float32).view(np.uint32)
    rounded = u + 0x7FFF + ((u >> 16) & 1)
    return (rounded >> 16).astype(np.uint16)


def make_consts(w_proj, b_proj):
    w = np.asarray(w_proj, dtype=np.float32)
    b = np.asarray(b_proj, dtype=np.float32)
    ws = np.empty((128, 16), dtype=np.float32)
    for i in range(4):
        for o in range(4):
            ws[:, 4 * i + o] = w[o, i] / C
    cbu = np.zeros((128, 532), dtype=np.float32)
    cbu[:, 0:16] = ws
    cbu[0:4, 16:20] = 1.0
    cbu[0:4, 20:532] = np.repeat(np.eye(4, dtype=np.float32), 128, axis=1)
    cf = np.zeros((128, 1), dtype=np.float32)
    cf[0:4, 0] = b
    return _to_bf16_bits(cbu), cf


LAST_RESULT = None


def kernel(g0, g1, g2, g3, w_proj, b_proj):
    global LAST_RESULT
    nc = _get_program()

    cbu, cf = make_consts(w_proj, b_proj)

    gall = np.stack(
        [np.asarray(x, dtype=np.float32).reshape(B, C, HW) for x in (g0, g1, g2, g3)],
        axis=1,
    )  # (B, 4, C, HW)
    in_maps = []
    for bi in range(NCORES):
        m = {"gall": np.ascontiguousarray(gall[bi]), "cbu": cbu, "cf": cf}
        in_maps.append(m)

    res = run_bass_kernel_spmd(
        nc,
        in_maps,
        list(range(NCORES)),
        trace=bool(int(os.environ.get("CM_TRACE", "0"))),
        tmpdir=os.environ.get("CM_TRACE_DIR") or None,
    )
    LAST_RESULT = res
    out_full = np.stack(
        [res.results[bi]["out"].reshape(C, H, W) for bi in range(NCORES)], axis=0
    )
    return out_full
